# revision 26
# baseline (speedup 1.0000x reference)
"""Causal self-attention (B=2, T=2048, C=1024, H=16) on 8 TRN2 NeuronCores.

Sharding: data-parallel over batch x tensor-parallel over heads.
Core c handles batch c//4 and the 4 heads (c%4)*4 .. (c%4)*4+3:
  - QKV projection restricted to its heads' columns of W_attn
  - per-head causal attention (scores kept transposed: ST[j, i])
  - softmax denominator obtained by augmenting V with a ones column,
    so P@V and the row sums come from the same matmuls
  - row-parallel output projection with its heads' rows of W_proj
Host sums the 4 partial projections per batch and adds b_proj.

v2: x is pre-transposed on the host (no PE transposes / DVE copies).
The whole kernel is one modulo-software-pipelined stream over four
512-column chunks: during chunk ci's PV phase, h0 zips with ci's
pair-(2,3) scores, h1 zips the next chunk's QKV + previous chunk's
out-proj, and h2/h3 emit the NEXT chunk's pair-(0,1) scores, so the
Act engine (exp, the secondary bottleneck at ~0.85ns/col + 343ns/op)
never runs dry. Score tiles pack two j-blocks [128,1024] (diagonal
blocks compacted) to halve exp instruction count; the two heads of a
pair occupy PE row-quadrants 0:64/64:128 and their score matmuls are
interleaved for quadrant concurrency. Causal masking multiplies only
the 128x128 diagonal triangle in-place on Pool; softmax denominators
ride along as a ones-column of V; normalization is recip + ones-bcast
matmul; projection partials return as bf16 (halves output DMA).
"""
import os
import sys
sys.path.insert(0, '/opt/trn_rl_repo')
os.environ.setdefault("JAX_PLATFORMS", "axon,cpu")

from contextlib import ExitStack

import numpy as np

import concourse.bass as bass
import concourse.tile as tile
from concourse import library_config, mybir

B, T, C, H, HD = 2, 2048, 1024, 16, 64
N_CORES = 8
HPC = H // (N_CORES // B)     # heads per core = 4
CPH = HPC * HD                # channel slice per core = 256

f32 = mybir.dt.float32
f32r = mybir.dt.float32r
bf16 = mybir.dt.bfloat16
AF = mybir.ActivationFunctionType

# ---------------------------------------------------------------------------
# Workaround for this container's walrus codegen, which rejects instructions
# carrying more than one sync-wait command ("Too many sync wait commands").
# After Tile scheduling, hoist excess waits onto same-engine NoOps inserted
# immediately before the owning instruction (engine streams are sequential,
# so this preserves semantics exactly).
# ---------------------------------------------------------------------------
import concourse.tile as tile_mod
from bass_rust import ScopedClock, SyncInfo

MAX_WAITS = 1


def _drain_and_barrier(self, tick_clock, wait_clock):
    nc = self.nc
    drain_inst = nc.sync.drain()
    wait_clock.add_sem_waits(
        drain_inst.ins, ScopedClock({None: tick_clock.global_clock})
    )
    si = drain_inst.ins.sync_info
    if si is not None and len(si.on_wait) > MAX_WAITS:
        waits = list(si.on_wait)
        drain_inst.ins.sync_info = SyncInfo(
            on_wait=waits[:MAX_WAITS], on_update=list(si.on_update)
        )
        for k in range(MAX_WAITS, len(waits), MAX_WAITS):
            nop = nc.sync.nop(nofuse=True)
            nop.ins.sync_info = SyncInfo(on_wait=waits[k:k + MAX_WAITS], on_update=[])
    nc.all_engine_barrier()
    assert self.sems is not None
    popped = nc._tile_sem_poison_stack.pop()
    assert popped is self._sem_poison
    nc.clear_and_free_semaphores(list(self.sems.allocated().values()))
    nc.all_engine_barrier()


tile_mod.TileContext._drain_and_barrier = _drain_and_barrier

_split_counter = [0]


def split_excess_waits(nc, max_waits=MAX_WAITS):
    n_split = 0
    for f in nc.m.functions:
        for bb in f.blocks:
            il = bb.instructions
            out = []
            for ins in il:
                si = ins.sync_info
                if si is not None and len(si.on_wait) > max_waits:
                    waits = list(si.on_wait)
                    extra = waits[:-max_waits]
                    for k in range(0, len(extra), max_waits):
                        _split_counter[0] += 1
                        nop = mybir.InstNoOp(
                            name=f"wsplit-{_split_counter[0]}", ins=[], outs=[]
                        )
                        nop.engine = ins.engine
                        nop.sync_info = SyncInfo(
                            on_wait=extra[k:k + max_waits], on_update=[]
                        )
                        out.append(nop)
                    ins.sync_info = SyncInfo(
                        on_wait=waits[-max_waits:], on_update=list(si.on_update)
                    )
                    n_split += 1
                out.append(ins)
            if len(out) != len(il):
                il[:] = out
    return n_split


# ---------------------------------------------------------------------------
# Program builder
# ---------------------------------------------------------------------------
def build_program(reps=1, split_waits=True, phases="abcd"):
    nc = bass.Bass("TRN2", target_bir_lowering=False, debug=False)

    xt_d = nc.dram_tensor("xt", [C, T], bf16, kind="ExternalInput")
    wqkv_d = nc.dram_tensor("wqkv", [C, 3 * CPH], bf16, kind="ExternalInput")
    bqk_d = nc.dram_tensor("bqk", [128, 4], f32, kind="ExternalInput")
    bv_d = nc.dram_tensor("bv", [128, CPH], f32, kind="ExternalInput")
    wp_d = nc.dram_tensor("wp", [CPH, C], bf16, kind="ExternalInput")
    trimask_d = nc.dram_tensor("trimask", [128, 128], bf16, kind="ExternalInput")
    y_d = nc.dram_tensor("y", [T, C], bf16, kind="ExternalOutput")

    NT = T // 128    # 16 t-blocks
    NCB = C // 128   # 8 c-blocks
    NI = T // 512    # 4 i-chunks

    with tile.TileContext(nc) as tc:
        with ExitStack() as ctx:
            const = ctx.enter_context(tc.tile_pool(name="const", bufs=1))
            trimask_t = const.tile([128, 128], bf16, tag="trimask")
            nc.sync.dma_start(trimask_t[:], trimask_d.ap())
            bqk_t = const.tile([128, 4], f32, tag="bqk")
            nc.sync.dma_start(bqk_t[:], bqk_d.ap())
            bv_t = const.tile([128, CPH], f32, tag="bv")
            nc.sync.dma_start(bv_t[:], bv_d.ap())
            ones4_t = const.tile([128, 4], f32, tag="ones4")
            nc.gpsimd.memset(ones4_t[:], 1.0)
            tones_f = const.tile([1, 64], f32, tag="tones_f")
            nc.gpsimd.memset(tones_f[:], 1.0)
            tones_t = const.tile([1, 64], f32r, tag="tones")
            nc.vector.tensor_copy(tones_t[:], tones_f[:])

            def body():
                with ExitStack() as c2:
                    # ---- persistent SBUF -----------------------------------
                    xw_p = c2.enter_context(tc.tile_pool(name="xw", bufs=1))
                    qk_p = c2.enter_context(tc.tile_pool(name="qk", bufs=1))
                    va_p = c2.enter_context(tc.tile_pool(name="va", bufs=1))
                    yt_p = c2.enter_context(tc.tile_pool(name="yt", bufs=1))
                    xt = [xw_p.tile([128, T], bf16, tag=f"xt{cb}", name=f"xt{cb}")
                          for cb in range(NCB)]
                    wt = [xw_p.tile([128, 3 * CPH], bf16, tag=f"wt{cb}",
                                    name=f"wt{cb}") for cb in range(NCB)]
                    wpt = [xw_p.tile([128, C], bf16, tag=f"wp{kb}",
                                     name=f"wpt{kb}") for kb in range(2)]
                    # qkt[0..1]: Q^T two heads per tile; qkt[2..3]: K^T
                    qkt = [qk_p.tile([128, T], bf16, tag=f"qkt{m}", name=f"qkt{m}")
                           for m in range(4)]
                    # V augmented with a ones column per head: [128, 4*65]
                    vaug = [va_p.tile([128, HPC * 65], bf16, tag=f"va{tb}",
                                      name=f"va{tb}") for tb in range(NT)]
                    # normalized Y^T, two heads stacked per tile
                    yts = [yt_p.tile([128, T], bf16, tag=f"yts{k}", name=f"yts{k}")
                           for k in range(2)]

                    def dma_chunk(ci):
                        csl = slice(ci * 512, ci * 512 + 512)
                        for cb in range(NCB):
                            nc.sync.dma_start(
                                xt[cb][:, csl],
                                xt_d.ap()[cb * 128:(cb + 1) * 128, csl])

                    # interleave per-cb so the first QKV accumulation chain
                    # can start as soon as each (wt, xt) pair lands
                    for cb in range(NCB):
                        nc.sync.dma_start(
                            wt[cb][:, 0:384],
                            wqkv_d.ap()[cb * 128:(cb + 1) * 128, 0:384])
                        nc.sync.dma_start(
                            xt[cb][:, 0:512],
                            xt_d.ap()[cb * 128:(cb + 1) * 128, 0:512])
                    for cb in range(NCB):
                        nc.sync.dma_start(
                            wt[cb][:, 384:768],
                            wqkv_d.ap()[cb * 128:(cb + 1) * 128, 384:768])
                    for kb in range(2):
                        nc.sync.dma_start(wpt[kb][:],
                                          wp_d.ap()[kb * 128:(kb + 1) * 128, :])

                    with ExitStack() as c3:
                        genps = c3.enter_context(
                            tc.tile_pool(name="genps", bufs=2, space="PSUM"))
                        sps = c3.enter_context(
                            tc.tile_pool(name="sps", bufs=2, space="PSUM"))
                        yps = c3.enter_context(
                            tc.tile_pool(name="yps", bufs=2, space="PSUM"))
                        ep = c3.enter_context(tc.tile_pool(name="ep", bufs=20))
                        rp = c3.enter_context(tc.tile_pool(name="rp", bufs=8))
                        op = c3.enter_context(tc.tile_pool(name="op", bufs=8))

                        if "c" not in phases:
                            for k2 in range(2):
                                nc.vector.memset(yts[k2][:], 0.0)

                        # ---- emission helpers ------------------------------
                        def qk_mm(ci, m):
                            isl = slice(ci * 512, ci * 512 + 512)
                            ps = genps.tile([128, 512], f32, tag="gen")
                            for cb in range(NCB):
                                nc.tensor.matmul(
                                    ps[:],
                                    lhsT=wt[cb][:, m * 128:(m + 1) * 128],
                                    rhs=xt[cb][:, isl],
                                    start=(cb == 0), stop=(cb == NCB - 1))
                            nc.vector.tensor_scalar_add(
                                qkt[m][:, isl], ps[:], bqk_t[:, m:m + 1])

                        def v_mm(tb):
                            ps = genps.tile([128, 512], f32, tag="gen")
                            for cb in range(NCB):
                                nc.tensor.matmul(
                                    ps[:, 0:CPH],
                                    lhsT=xt[cb][:, tb * 128:(tb + 1) * 128],
                                    rhs=wt[cb][:, 2 * CPH:3 * CPH],
                                    start=(cb == 0), stop=(cb == NCB - 1))
                            vv = vaug[tb][:].rearrange("p (h e) -> p h e", e=65)
                            nc.vector.tensor_add(
                                vv[:, :, 0:64],
                                ps[:, 0:CPH].rearrange("p (h d) -> p h d", d=64),
                                bv_t[:].rearrange("p (h d) -> p h d", d=64))
                            nc.gpsimd.tensor_copy(
                                vv[:, :, 64:65],
                                ones4_t[:].rearrange("p (h e) -> p h e", e=1))

                        # j-block layout inside a score tile for logical
                        # pair index p of chunk ci: full pairs pack j-blocks
                        # (2p, 2p+1) at columns 0/512; the two diagonal
                        # "pairs" pack (4ci, 4ci+2) at columns 0/512 and
                        # (4ci+1, 4ci+3) at columns 0/384 (compacted so one
                        # exp covers the whole used range).
                        def pair_blocks(ci, p):
                            """[(bj, tile_col, lo), ...] for pair p; plus the
                            number of used columns."""
                            if p < 2 * ci:
                                return [(2 * p, 0, 0), (2 * p + 1, 512, 0)], 1024
                            if p == 2 * ci:      # diag A: k=0 and k=2
                                return [(4 * ci, 0, 0),
                                        (4 * ci + 2, 512, 256)], 768
                            # diag B: k=1 and k=3
                            return [(4 * ci + 1, 0, 128),
                                    (4 * ci + 3, 384, 384)], 512

                        def s_halfpair(ci, hp, p):
                            """Scores for head pair (2hp, 2hp+1), logical pair
                            p vs i-chunk ci. The two heads sit in row quadrants
                            0:64 / 64:128 of the qkt tiles, so alternating
                            their matmuls lets the PE run both quadrants
                            concurrently. Returns (es_lo, es_hi)."""
                            isl = slice(ci * 512, ci * 512 + 512)
                            qt_t = qkt[hp]
                            kt_t = qkt[2 + hp]
                            blocks, used = pair_blocks(ci, p)
                            sts, ess = [], []
                            for sub in range(2):
                                sts.append(sps.tile([128, 1024], f32, tag="st", name=f"st{sub}"))
                                ess.append(ep.tile([128, 1024], bf16, tag="es", name=f"es{sub}"))
                            for (bj, col, lo) in blocks:
                                jsl = slice(bj * 128, bj * 128 + 128)
                                osl = slice(col, col + 512 - lo)
                                for sub in range(2):   # alternate quadrants
                                    prow = slice(sub * 64, sub * 64 + 64)
                                    nc.tensor.matmul(
                                        sts[sub][:, osl],
                                        lhsT=kt_t[prow, jsl],
                                        rhs=qt_t[prow, isl][:, lo:],
                                        start=True, stop=True)
                            for sub in range(2):
                                st, es = sts[sub], ess[sub]
                                nc.scalar.activation(
                                    es[:, 0:used], st[:, 0:used], AF.Exp,
                                    scale=0.125)
                                if p >= 2 * ci:     # mask diag triangles
                                    for (bj, col, lo) in blocks:
                                        msl = slice(col, col + 128)
                                        nc.gpsimd.tensor_mul(
                                            es[:, msl], es[:, msl],
                                            trimask_t[:])
                            return ess

                        def pv_pair(ci, h, p, yt, es):
                            blocks, _ = pair_blocks(ci, p)
                            for (bj, col, lo) in blocks:
                                nc.tensor.matmul(
                                    yt[0:65, lo:],
                                    lhsT=vaug[bj][:, h * 65:(h + 1) * 65],
                                    rhs=es[:, col:col + 512 - lo],
                                    start=(bj == 0), stop=(bj == 4 * ci + 3))

                        def norm(ci, h, yt):
                            """recip + ones-bcast matmul + scale into yts."""
                            isl = slice(ci * 512, ci * 512 + 512)
                            prow = slice((h % 2) * 64, (h % 2) * 64 + 64)
                            rc = rp.tile([1, 512], f32r, tag="rc")
                            with nc.allow_low_precision(
                                    reason="f32r operand for bcast matmul"):
                                nc.vector.reciprocal(rc[:], yt[64:65, :])
                            bc = genps.tile([128, 512], f32, tag="gen")
                            nc.tensor.matmul(bc[0:64, :], lhsT=tones_t[:],
                                             rhs=rc[:], start=True, stop=True)
                            bs = rp.tile([64, 512], f32, tag="bs")
                            nc.vector.tensor_copy(bs[:], bc[0:64, :])
                            nc.vector.tensor_mul(
                                yts[h // 2][prow, isl], yt[0:64, :], bs[:])

                        def proj_block(tb, nn_):
                            ps = genps.tile([128, 512], f32, tag="gen")
                            for kb in range(2):
                                nc.tensor.matmul(
                                    ps[:],
                                    lhsT=yts[kb][:, tb * 128:(tb + 1) * 128],
                                    rhs=wpt[kb][:, nn_ * 512:(nn_ + 1) * 512],
                                    start=(kb == 0), stop=(kb == 1))
                            ob = op.tile([128, 512], bf16, tag="ob")
                            nc.vector.tensor_copy(ob[:], ps[:])
                            nc.sync.dma_start(
                                y_d.ap()[tb * 128:(tb + 1) * 128,
                                         nn_ * 512:(nn_ + 1) * 512],
                                ob[:])

                        def proj(ci):
                            for tb in range(4 * ci, 4 * ci + 4):
                                for nn_ in range(2):
                                    proj_block(tb, nn_)

                        # ---- interleaved emission --------------------------
                        # Modulo software pipeline across chunks. During chunk
                        # ci's PV phase: h0 zips with ci's pair-(2,3) scores,
                        # h1 zips next chunk's QKV + previous chunk's proj,
                        # h2/h3 zip the NEXT chunk's pair-(0,1) scores, so the
                        # Act engine (exp) never runs dry between chunks.
                        from collections import deque
                        filler = deque()

                        def fill(n):
                            for _ in range(min(n, len(filler))):
                                filler.popleft()()

                        pend_norm = None     # (ci, h, yt) awaiting emission
                        es01 = None
                        for ci in range(NI):
                            npairs = 2 * ci + 2
                            if ci + 1 < NI:
                                dma_chunk(ci + 1)
                            if "c" not in phases:
                                if ci == 0:
                                    for m in (0, 2, 1, 3):
                                        qk_mm(0, m)
                                    for tb in range(0, 4):
                                        v_mm(tb)
                                else:
                                    fill(len(filler))
                                if "d" in phases:
                                    proj(ci)
                                if ci + 1 < NI:
                                    for m in (0, 2, 1, 3):
                                        filler.append(
                                            lambda ci=ci, m=m: qk_mm(ci + 1, m))
                                    for tb in range(4 * ci + 4, 4 * ci + 8):
                                        filler.append(
                                            lambda tb=tb: v_mm(tb))
                                continue

                            if ci == 0:
                                # prologue: chunk 0 QKV + pair-(0,1) scores
                                qk_mm(0, 0)
                                qk_mm(0, 2)
                                es01 = []
                                for p in range(npairs):
                                    es01.append(s_halfpair(0, 0, p))
                                    if p == 0:
                                        qk_mm(0, 1)
                                        qk_mm(0, 3)
                                    v_mm(2 * p)
                                    v_mm(2 * p + 1)

                            # PV h0 zipped with this chunk's pair-(2,3) scores
                            es23 = []
                            yt0 = yps.tile([128, 512], f32, tag="yt")
                            for p in range(npairs):
                                es23.append(s_halfpair(ci, 1, p))
                                pv_pair(ci, 0, p, yt0, es01[p][0])
                                if p == 0 and pend_norm is not None:
                                    norm(*pend_norm)
                                    pend_norm = None
                            pend_norm = (ci, 0, yt0)

                            # queue fillers: next chunk's QKV, prev chunk proj
                            if ci + 1 < NI:
                                for m in (0, 2, 1, 3):
                                    filler.append(
                                        lambda ci=ci, m=m: qk_mm(ci + 1, m))
                                for tb in range(4 * ci + 4, 4 * ci + 8):
                                    filler.append(lambda tb=tb: v_mm(tb))
                            if "d" in phases and ci > 0:
                                for tb in range(4 * ci - 4, 4 * ci):
                                    for nn_ in range(2):
                                        filler.append(
                                            lambda tb=tb, nn_=nn_:
                                            proj_block(tb, nn_))

                            # PV h1: zip fillers (QKV of ci+1 must drain here)
                            yt = yps.tile([128, 512], f32, tag="yt")
                            for p in range(npairs):
                                pv_pair(ci, 1, p, yt, es01[p][1])
                                fill(4)
                                if p == npairs // 2 and pend_norm is not None:
                                    norm(*pend_norm)
                                    pend_norm = None
                            if pend_norm is not None:
                                norm(*pend_norm)
                            pend_norm = (ci, 1, yt)
                            fill(len(filler) - 8 if ci + 1 < NI else 0)

                            # PV h2/h3: zip NEXT chunk's pair-(0,1) scores
                            es01_next = []
                            np_next = 2 * ci + 4
                            for h in (2, 3):
                                yt = yps.tile([128, 512], f32, tag="yt")
                                for p in range(npairs):
                                    if ci + 1 < NI and len(es01_next) < np_next:
                                        want = (np_next * (p + 1 +
                                                (h - 2) * npairs) +
                                                2 * npairs - 1) // (2 * npairs)
                                        while len(es01_next) < want:
                                            es01_next.append(
                                                s_halfpair(ci + 1, 0,
                                                           len(es01_next)))
                                    pv_pair(ci, h, p, yt, es23[p][h % 2])
                                    fill(2)
                                    if p == npairs // 2 and pend_norm is not None:
                                        norm(*pend_norm)
                                        pend_norm = None
                                if pend_norm is not None:
                                    norm(*pend_norm)
                                pend_norm = (ci, h, yt)
                            while ci + 1 < NI and len(es01_next) < np_next:
                                es01_next.append(
                                    s_halfpair(ci + 1, 0, len(es01_next)))
                            fill(len(filler))
                            es01 = es01_next
                        if pend_norm is not None:
                            norm(*pend_norm)
                        if "c" in phases and "d" in phases:
                            proj(NI - 1)

            if reps == 1:
                body()
            else:
                with tc.For_i(0, reps, 1, hint_engines=(
                        mybir.EngineType.PE, mybir.EngineType.Activation,
                        mybir.EngineType.DVE, mybir.EngineType.SP,
                        mybir.EngineType.Pool)):
                    body()

    if split_waits:
        split_excess_waits(nc)
    return nc


# ---------------------------------------------------------------------------
# Cached PJRT runner (fork of concourse.bass2jax.run_bass_via_pjrt that keeps
# the jitted executable so repeat kernel() calls don't recompile)
# ---------------------------------------------------------------------------
_RUNNERS = {}


def _make_pjrt(nc, donate=True, tag="main"):
    import jax
    from jax.sharding import Mesh, PartitionSpec
    from jax.experimental.shard_map import shard_map
    from concourse import bass2jax as b2j

    b2j.install_neuronx_cc_hook()

    partition_name = (
        nc.partition_id_tensor.name if nc.partition_id_tensor else None
    )
    in_names, out_names, out_avals, zero_outs = [], [], [], []
    for alloc in nc.m.functions[0].allocations:
        if not isinstance(alloc, mybir.MemoryLocationSet):
            continue
        name = alloc.memorylocations[0].name
        if alloc.kind == "ExternalInput":
            if name != partition_name:
                in_names.append(name)
        elif alloc.kind == "ExternalOutput":
            out_names.append(name)
            shape = tuple(alloc.tensor_shape)
            dtype = mybir.dt.np(alloc.dtype)
            out_avals.append(jax.core.ShapedArray(shape, dtype))
            zero_outs.append(np.zeros(shape, dtype))
    n_params = len(in_names)
    n_outs = len(out_avals)
    all_names = in_names + out_names
    if partition_name is not None:
        all_names = all_names + [partition_name]
    donate_idx = tuple(range(n_params, n_params + n_outs))

    def _body(*args):
        operands = list(args)
        if partition_name is not None:
            operands.append(b2j.partition_id_tensor())
        outs = b2j._bass_exec_p.bind(
            *operands,
            out_avals=tuple(out_avals),
            in_names=tuple(all_names),
            out_names=tuple(out_names),
            lowering_input_output_aliases=(),
            sim_require_finite=True,
            sim_require_nnan=True,
            nc=nc,
        )
        return tuple(outs)

    _body.__name__ = f"_body_{tag}"
    _body.__qualname__ = f"_body_{tag}"

    devices = jax.devices()[:N_CORES]
    mesh = Mesh(np.asarray(devices), ("core",))
    in_specs = (PartitionSpec("core"),) * (n_params + n_outs)
    out_specs = (PartitionSpec("core"),) * n_outs
    sharded = jax.jit(
        shard_map(_body, mesh=mesh, in_specs=in_specs, out_specs=out_specs,
                  check_rep=False),
        donate_argnums=donate_idx if donate else (), keep_unused=True)

    def concat_args(in_maps):
        per_core = [[np.asarray(m[name]) for name in in_names] for m in in_maps]
        concat_in = [
            np.concatenate([per_core[c][i] for c in range(N_CORES)], axis=0)
            for i in range(n_params)
        ]
        concat_zeros = [
            np.zeros((N_CORES * z.shape[0], *z.shape[1:]), z.dtype)
            for z in zero_outs
        ]
        return concat_in + concat_zeros

    def run(in_maps):
        out_arrs = sharded(*concat_args(in_maps))
        return [
            {name: np.asarray(out_arrs[i]).reshape(N_CORES, *out_avals[i].shape)[c]
             for i, name in enumerate(out_names)}
            for c in range(N_CORES)
        ]

    info = {
        "sharded": sharded, "concat_args": concat_args, "mesh": mesh,
        "PartitionSpec": PartitionSpec, "jax": jax,
    }
    return run, info


def _get_runner(key, nc):
    if key in _RUNNERS:
        return _RUNNERS[key]
    run, _ = _make_pjrt(nc, donate=True, tag=key)
    _RUNNERS[key] = run
    return run


def get_timed_runner(nc, tag="timed"):
    """No donation, device-resident args: returns (call, dev_args_fn)."""
    run, info = _make_pjrt(nc, donate=False, tag=tag)
    jax = info["jax"]
    sharding = jax.sharding.NamedSharding(
        info["mesh"], info["PartitionSpec"]("core"))

    def prepare(in_maps):
        return [jax.device_put(a, sharding) for a in info["concat_args"](in_maps)]

    def call(dev_args):
        outs = info["sharded"](*dev_args)
        jax.block_until_ready(outs)
        return outs

    return prepare, call


# ---------------------------------------------------------------------------
# Host-side sharding / gathering
# ---------------------------------------------------------------------------
def make_in_maps(x, W_attn, b_attn, W_proj):
    import ml_dtypes
    rj = np.arange(128)[:, None]
    ri = np.arange(128)[None, :]
    trimask = (rj <= ri).astype(ml_dtypes.bfloat16)
    in_maps = []
    for c in range(N_CORES):
        b = c // (N_CORES // B)
        g = c % (N_CORES // B)
        cs = slice(CPH * g, CPH * g + CPH)
        wq = W_attn[:, CPH * g:CPH * g + CPH]
        wk = W_attn[:, C + CPH * g:C + CPH * g + CPH]
        wv = W_attn[:, 2 * C + CPH * g:2 * C + CPH * g + CPH]
        wqkv = np.ascontiguousarray(
            np.concatenate([wq, wk, wv], axis=1).astype(ml_dtypes.bfloat16))
        bq = b_attn[CPH * g:CPH * g + CPH]
        bk = b_attn[C + CPH * g:C + CPH * g + CPH]
        bvv = b_attn[2 * C + CPH * g:2 * C + CPH * g + CPH]
        bqk = np.ascontiguousarray(
            np.stack([bq[:128], bq[128:], bk[:128], bk[128:]], axis=1))
        bv_arr = np.ascontiguousarray(
            np.broadcast_to(bvv[None, :], (128, CPH)))
        wp = np.ascontiguousarray(W_proj[cs, :].astype(ml_dtypes.bfloat16))
        in_maps.append({
            "xt": np.ascontiguousarray(x[b].T.astype(ml_dtypes.bfloat16)),
            "wqkv": wqkv, "bqk": bqk, "bv": bv_arr, "wp": wp,
            "trimask": trimask,
        })
    return in_maps


def kernel(x, W_attn, b_attn, W_proj, b_proj):
    x = np.asarray(x, dtype=np.float32)
    W_attn = np.asarray(W_attn, dtype=np.float32)
    b_attn = np.asarray(b_attn, dtype=np.float32)
    W_proj = np.asarray(W_proj, dtype=np.float32)
    b_proj = np.asarray(b_proj, dtype=np.float32)

    if "main" not in _RUNNERS:
        nc = build_program(reps=1)
        run = _get_runner("main", nc)
    else:
        run = _RUNNERS["main"]

    results = run(make_in_maps(x, W_attn, b_attn, W_proj))

    out = np.empty((B, T, C), dtype=np.float32)
    gpb = N_CORES // B
    for b in range(B):
        acc = results[gpb * b]["y"].astype(np.float32)
        for g in range(1, gpb):
            acc = acc + results[gpb * b + g]["y"].astype(np.float32)
        out[b] = acc + b_proj[None, :]
    return out


# revision 27
# speedup vs baseline: 1.0271x; 1.0271x over previous
"""Causal self-attention (B=2, T=2048, C=1024, H=16) on 8 TRN2 NeuronCores.

Sharding: data-parallel over batch x tensor-parallel over heads.
Core c handles batch c//4 and the 4 heads (c%4)*4 .. (c%4)*4+3:
  - QKV projection restricted to its heads' columns of W_attn
  - per-head causal attention (scores kept transposed: ST[j, i])
  - softmax denominator obtained by augmenting V with a ones column,
    so P@V and the row sums come from the same matmuls
  - row-parallel output projection with its heads' rows of W_proj
Host sums the 4 partial projections per batch and adds b_proj.

v2: x is pre-transposed on the host (no PE transposes / DVE copies).
The whole kernel is one modulo-software-pipelined stream over four
512-column chunks: during chunk ci's PV phase, h0 zips with ci's
pair-(2,3) scores, h1 zips the next chunk's QKV + previous chunk's
out-proj, and h2/h3 emit the NEXT chunk's pair-(0,1) scores, so the
Act engine (exp, the secondary bottleneck at ~0.85ns/col + 343ns/op)
never runs dry. Score tiles pack two j-blocks [128,1024] (diagonal
blocks compacted) to halve exp instruction count; the two heads of a
pair occupy PE row-quadrants 0:64/64:128 and their score matmuls are
interleaved for quadrant concurrency. Causal masking multiplies only
the 128x128 diagonal triangle in-place on Pool; softmax denominators
ride along as a ones-column of V; normalization is recip + ones-bcast
matmul; projection partials return as bf16 (halves output DMA).
"""
import os
import sys
sys.path.insert(0, '/opt/trn_rl_repo')
os.environ.setdefault("JAX_PLATFORMS", "axon,cpu")

from contextlib import ExitStack

import numpy as np

import concourse.bass as bass
import concourse.tile as tile
from concourse import library_config, mybir

B, T, C, H, HD = 2, 2048, 1024, 16, 64
N_CORES = 8
HPC = H // (N_CORES // B)     # heads per core = 4
CPH = HPC * HD                # channel slice per core = 256

f32 = mybir.dt.float32
f32r = mybir.dt.float32r
bf16 = mybir.dt.bfloat16
AF = mybir.ActivationFunctionType

# ---------------------------------------------------------------------------
# Workaround for this container's walrus codegen, which rejects instructions
# carrying more than one sync-wait command ("Too many sync wait commands").
# After Tile scheduling, hoist excess waits onto same-engine NoOps inserted
# immediately before the owning instruction (engine streams are sequential,
# so this preserves semantics exactly).
# ---------------------------------------------------------------------------
import concourse.tile as tile_mod
from bass_rust import ScopedClock, SyncInfo

MAX_WAITS = 1


def _drain_and_barrier(self, tick_clock, wait_clock):
    nc = self.nc
    drain_inst = nc.sync.drain()
    wait_clock.add_sem_waits(
        drain_inst.ins, ScopedClock({None: tick_clock.global_clock})
    )
    si = drain_inst.ins.sync_info
    if si is not None and len(si.on_wait) > MAX_WAITS:
        waits = list(si.on_wait)
        drain_inst.ins.sync_info = SyncInfo(
            on_wait=waits[:MAX_WAITS], on_update=list(si.on_update)
        )
        for k in range(MAX_WAITS, len(waits), MAX_WAITS):
            nop = nc.sync.nop(nofuse=True)
            nop.ins.sync_info = SyncInfo(on_wait=waits[k:k + MAX_WAITS], on_update=[])
    nc.all_engine_barrier()
    assert self.sems is not None
    popped = nc._tile_sem_poison_stack.pop()
    assert popped is self._sem_poison
    nc.clear_and_free_semaphores(list(self.sems.allocated().values()))
    nc.all_engine_barrier()


tile_mod.TileContext._drain_and_barrier = _drain_and_barrier

_split_counter = [0]


def split_excess_waits(nc, max_waits=MAX_WAITS):
    n_split = 0
    for f in nc.m.functions:
        for bb in f.blocks:
            il = bb.instructions
            out = []
            for ins in il:
                si = ins.sync_info
                if si is not None and len(si.on_wait) > max_waits:
                    waits = list(si.on_wait)
                    extra = waits[:-max_waits]
                    for k in range(0, len(extra), max_waits):
                        _split_counter[0] += 1
                        nop = mybir.InstNoOp(
                            name=f"wsplit-{_split_counter[0]}", ins=[], outs=[]
                        )
                        nop.engine = ins.engine
                        nop.sync_info = SyncInfo(
                            on_wait=extra[k:k + max_waits], on_update=[]
                        )
                        out.append(nop)
                    ins.sync_info = SyncInfo(
                        on_wait=waits[-max_waits:], on_update=list(si.on_update)
                    )
                    n_split += 1
                out.append(ins)
            if len(out) != len(il):
                il[:] = out
    return n_split


# ---------------------------------------------------------------------------
# Program builder
# ---------------------------------------------------------------------------
def build_program(reps=1, split_waits=True, phases="abcd"):
    nc = bass.Bass("TRN2", target_bir_lowering=False, debug=False)

    xt_d = nc.dram_tensor("xt", [C, T], bf16, kind="ExternalInput")
    wqkv_d = nc.dram_tensor("wqkv", [C, 3 * CPH], bf16, kind="ExternalInput")
    bqk_d = nc.dram_tensor("bqk", [128, 4], f32, kind="ExternalInput")
    bv_d = nc.dram_tensor("bv", [128, CPH], f32, kind="ExternalInput")
    wp_d = nc.dram_tensor("wp", [CPH, C], bf16, kind="ExternalInput")
    trimask_d = nc.dram_tensor("trimask", [128, 128], bf16, kind="ExternalInput")
    y_d = nc.dram_tensor("y", [T, C], bf16, kind="ExternalOutput")

    NT = T // 128    # 16 t-blocks
    NCB = C // 128   # 8 c-blocks
    NI = T // 512    # 4 i-chunks

    with tile.TileContext(nc) as tc:
        with ExitStack() as ctx:
            const = ctx.enter_context(tc.tile_pool(name="const", bufs=1))
            trimask_t = const.tile([128, 128], bf16, tag="trimask")
            nc.sync.dma_start(trimask_t[:], trimask_d.ap())
            bqk_t = const.tile([128, 4], f32, tag="bqk")
            nc.sync.dma_start(bqk_t[:], bqk_d.ap())
            bv_t = const.tile([128, CPH], f32, tag="bv")
            nc.sync.dma_start(bv_t[:], bv_d.ap())
            ones4_t = const.tile([128, 4], f32, tag="ones4")
            nc.gpsimd.memset(ones4_t[:], 1.0)
            tones_f = const.tile([1, 64], f32, tag="tones_f")
            nc.gpsimd.memset(tones_f[:], 1.0)
            tones_t = const.tile([1, 64], f32r, tag="tones")
            nc.vector.tensor_copy(tones_t[:], tones_f[:])

            def body():
                with ExitStack() as c2:
                    # ---- persistent SBUF -----------------------------------
                    xw_p = c2.enter_context(tc.tile_pool(name="xw", bufs=1))
                    qk_p = c2.enter_context(tc.tile_pool(name="qk", bufs=1))
                    va_p = c2.enter_context(tc.tile_pool(name="va", bufs=1))
                    yt_p = c2.enter_context(tc.tile_pool(name="yt", bufs=1))
                    xt = [xw_p.tile([128, T], bf16, tag=f"xt{cb}", name=f"xt{cb}")
                          for cb in range(NCB)]
                    wt = [xw_p.tile([128, 3 * CPH], bf16, tag=f"wt{cb}",
                                    name=f"wt{cb}") for cb in range(NCB)]
                    wpt = [xw_p.tile([128, C], bf16, tag=f"wp{kb}",
                                     name=f"wpt{kb}") for kb in range(2)]
                    # qkt[0..1]: Q^T two heads per tile; qkt[2..3]: K^T
                    qkt = [qk_p.tile([128, T], bf16, tag=f"qkt{m}", name=f"qkt{m}")
                           for m in range(4)]
                    # V augmented with a ones column per head: [128, 4*65]
                    vaug = [va_p.tile([128, HPC * 65], bf16, tag=f"va{tb}",
                                      name=f"va{tb}") for tb in range(NT)]
                    # normalized Y^T, two heads stacked per tile
                    yts = [yt_p.tile([128, T], bf16, tag=f"yts{k}", name=f"yts{k}")
                           for k in range(2)]

                    def dma_chunk(ci):
                        csl = slice(ci * 512, ci * 512 + 512)
                        for cb in range(NCB):
                            nc.sync.dma_start(
                                xt[cb][:, csl],
                                xt_d.ap()[cb * 128:(cb + 1) * 128, csl])

                    # interleave per-cb so the first QKV accumulation chain
                    # can start as soon as each (wt, xt) pair lands
                    for cb in range(NCB):
                        nc.sync.dma_start(
                            wt[cb][:, 0:384],
                            wqkv_d.ap()[cb * 128:(cb + 1) * 128, 0:384])
                        nc.sync.dma_start(
                            xt[cb][:, 0:512],
                            xt_d.ap()[cb * 128:(cb + 1) * 128, 0:512])
                    for cb in range(NCB):
                        nc.sync.dma_start(
                            wt[cb][:, 384:768],
                            wqkv_d.ap()[cb * 128:(cb + 1) * 128, 384:768])
                    for kb in range(2):
                        nc.sync.dma_start(wpt[kb][:],
                                          wp_d.ap()[kb * 128:(kb + 1) * 128, :])

                    with ExitStack() as c3:
                        genps = c3.enter_context(
                            tc.tile_pool(name="genps", bufs=2, space="PSUM"))
                        sps = c3.enter_context(
                            tc.tile_pool(name="sps", bufs=2, space="PSUM"))
                        yps = c3.enter_context(
                            tc.tile_pool(name="yps", bufs=2, space="PSUM"))
                        ep = c3.enter_context(tc.tile_pool(name="ep", bufs=20))
                        rp = c3.enter_context(tc.tile_pool(name="rp", bufs=8))
                        op = c3.enter_context(tc.tile_pool(name="op", bufs=8))

                        if "c" not in phases:
                            for k2 in range(2):
                                nc.vector.memset(yts[k2][:], 0.0)

                        # ---- emission helpers ------------------------------
                        def qk_mm(ci, m):
                            isl = slice(ci * 512, ci * 512 + 512)
                            ps = genps.tile([128, 512], f32, tag="gen")
                            for cb in range(NCB):
                                nc.tensor.matmul(
                                    ps[:],
                                    lhsT=wt[cb][:, m * 128:(m + 1) * 128],
                                    rhs=xt[cb][:, isl],
                                    start=(cb == 0), stop=(cb == NCB - 1))
                            nc.vector.tensor_scalar_add(
                                qkt[m][:, isl], ps[:], bqk_t[:, m:m + 1])

                        def v_mm(tb):
                            ps = genps.tile([128, 512], f32, tag="gen")
                            for cb in range(NCB):
                                nc.tensor.matmul(
                                    ps[:, 0:CPH],
                                    lhsT=xt[cb][:, tb * 128:(tb + 1) * 128],
                                    rhs=wt[cb][:, 2 * CPH:3 * CPH],
                                    start=(cb == 0), stop=(cb == NCB - 1))
                            vv = vaug[tb][:].rearrange("p (h e) -> p h e", e=65)
                            nc.vector.tensor_add(
                                vv[:, :, 0:64],
                                ps[:, 0:CPH].rearrange("p (h d) -> p h d", d=64),
                                bv_t[:].rearrange("p (h d) -> p h d", d=64))
                            nc.gpsimd.tensor_copy(
                                vv[:, :, 64:65],
                                ones4_t[:].rearrange("p (h e) -> p h e", e=1))

                        # j-block layout inside a score tile for logical
                        # pair index p of chunk ci: full pairs pack j-blocks
                        # (2p, 2p+1) at columns 0/512; the two diagonal
                        # "pairs" pack (4ci, 4ci+2) at columns 0/512 and
                        # (4ci+1, 4ci+3) at columns 0/384 (compacted so one
                        # exp covers the whole used range).
                        def pair_blocks(ci, p):
                            """[(bj, tile_col, lo), ...] for pair p; plus the
                            number of used columns."""
                            if p < 2 * ci:
                                return [(2 * p, 0, 0), (2 * p + 1, 512, 0)], 1024
                            if p == 2 * ci:      # diag A: k=0 and k=2
                                return [(4 * ci, 0, 0),
                                        (4 * ci + 2, 512, 256)], 768
                            # diag B: k=1 and k=3
                            return [(4 * ci + 1, 0, 128),
                                    (4 * ci + 3, 384, 384)], 512

                        def s_halfpair(ci, hp, p):
                            """Scores for head pair (2hp, 2hp+1), logical pair
                            p vs i-chunk ci. The two heads sit in row quadrants
                            0:64 / 64:128 of the qkt tiles, so alternating
                            their matmuls lets the PE run both quadrants
                            concurrently. Returns (es_lo, es_hi)."""
                            isl = slice(ci * 512, ci * 512 + 512)
                            qt_t = qkt[hp]
                            kt_t = qkt[2 + hp]
                            blocks, used = pair_blocks(ci, p)
                            sts, ess = [], []
                            for sub in range(2):
                                sts.append(sps.tile([128, 1024], f32, tag="st", name=f"st{sub}"))
                                ess.append(ep.tile([128, 1024], bf16, tag="es", name=f"es{sub}"))
                            for sub in range(2):   # A/B: sequential subs
                                prow = slice(sub * 64, sub * 64 + 64)
                                for (bj, col, lo) in blocks:
                                    jsl = slice(bj * 128, bj * 128 + 128)
                                    osl = slice(col, col + 512 - lo)
                                    nc.tensor.matmul(
                                        sts[sub][:, osl],
                                        lhsT=kt_t[prow, jsl],
                                        rhs=qt_t[prow, isl][:, lo:],
                                        start=True, stop=True)
                            for sub in range(2):
                                st, es = sts[sub], ess[sub]
                                nc.scalar.activation(
                                    es[:, 0:used], st[:, 0:used], AF.Exp,
                                    scale=0.125)
                                if p >= 2 * ci:     # mask diag triangles
                                    for (bj, col, lo) in blocks:
                                        msl = slice(col, col + 128)
                                        nc.gpsimd.tensor_mul(
                                            es[:, msl], es[:, msl],
                                            trimask_t[:])
                            return ess

                        def pv_pair(ci, h, p, yt, es):
                            blocks, _ = pair_blocks(ci, p)
                            for (bj, col, lo) in blocks:
                                nc.tensor.matmul(
                                    yt[0:65, lo:],
                                    lhsT=vaug[bj][:, h * 65:(h + 1) * 65],
                                    rhs=es[:, col:col + 512 - lo],
                                    start=(bj == 0), stop=(bj == 4 * ci + 3))

                        def norm(ci, h, yt):
                            """recip + ones-bcast matmul + scale into yts."""
                            isl = slice(ci * 512, ci * 512 + 512)
                            prow = slice((h % 2) * 64, (h % 2) * 64 + 64)
                            rc = rp.tile([1, 512], f32r, tag="rc")
                            with nc.allow_low_precision(
                                    reason="f32r operand for bcast matmul"):
                                nc.vector.reciprocal(rc[:], yt[64:65, :])
                            bc = genps.tile([128, 512], f32, tag="gen")
                            nc.tensor.matmul(bc[0:64, :], lhsT=tones_t[:],
                                             rhs=rc[:], start=True, stop=True)
                            bs = rp.tile([64, 512], f32, tag="bs")
                            nc.vector.tensor_copy(bs[:], bc[0:64, :])
                            nc.vector.tensor_mul(
                                yts[h // 2][prow, isl], yt[0:64, :], bs[:])

                        def proj_block(tb, nn_):
                            ps = genps.tile([128, 512], f32, tag="gen")
                            for kb in range(2):
                                nc.tensor.matmul(
                                    ps[:],
                                    lhsT=yts[kb][:, tb * 128:(tb + 1) * 128],
                                    rhs=wpt[kb][:, nn_ * 512:(nn_ + 1) * 512],
                                    start=(kb == 0), stop=(kb == 1))
                            ob = op.tile([128, 512], bf16, tag="ob")
                            nc.vector.tensor_copy(ob[:], ps[:])
                            nc.sync.dma_start(
                                y_d.ap()[tb * 128:(tb + 1) * 128,
                                         nn_ * 512:(nn_ + 1) * 512],
                                ob[:])

                        def proj(ci):
                            for tb in range(4 * ci, 4 * ci + 4):
                                for nn_ in range(2):
                                    proj_block(tb, nn_)

                        # ---- interleaved emission --------------------------
                        # Modulo software pipeline across chunks. During chunk
                        # ci's PV phase: h0 zips with ci's pair-(2,3) scores,
                        # h1 zips next chunk's QKV + previous chunk's proj,
                        # h2/h3 zip the NEXT chunk's pair-(0,1) scores, so the
                        # Act engine (exp) never runs dry between chunks.
                        from collections import deque
                        filler = deque()

                        def fill(n):
                            for _ in range(min(n, len(filler))):
                                filler.popleft()()

                        pend_norm = None     # (ci, h, yt) awaiting emission
                        es01 = None
                        for ci in range(NI):
                            npairs = 2 * ci + 2
                            if ci + 1 < NI:
                                dma_chunk(ci + 1)
                            if "c" not in phases:
                                if ci == 0:
                                    for m in (0, 2, 1, 3):
                                        qk_mm(0, m)
                                    for tb in range(0, 4):
                                        v_mm(tb)
                                else:
                                    fill(len(filler))
                                if "d" in phases:
                                    proj(ci)
                                if ci + 1 < NI:
                                    for m in (0, 2, 1, 3):
                                        filler.append(
                                            lambda ci=ci, m=m: qk_mm(ci + 1, m))
                                    for tb in range(4 * ci + 4, 4 * ci + 8):
                                        filler.append(
                                            lambda tb=tb: v_mm(tb))
                                continue

                            if ci == 0:
                                # prologue: chunk 0 QKV + pair-(0,1) scores
                                qk_mm(0, 0)
                                qk_mm(0, 2)
                                es01 = []
                                for p in range(npairs):
                                    es01.append(s_halfpair(0, 0, p))
                                    if p == 0:
                                        qk_mm(0, 1)
                                        qk_mm(0, 3)
                                    v_mm(2 * p)
                                    v_mm(2 * p + 1)

                            # PV h0 zipped with this chunk's pair-(2,3) scores
                            es23 = []
                            yt0 = yps.tile([128, 512], f32, tag="yt")
                            for p in range(npairs):
                                es23.append(s_halfpair(ci, 1, p))
                                pv_pair(ci, 0, p, yt0, es01[p][0])
                                if p == 0 and pend_norm is not None:
                                    norm(*pend_norm)
                                    pend_norm = None
                            pend_norm = (ci, 0, yt0)

                            # queue fillers: next chunk's QKV, prev chunk proj
                            if ci + 1 < NI:
                                for m in (0, 2, 1, 3):
                                    filler.append(
                                        lambda ci=ci, m=m: qk_mm(ci + 1, m))
                                for tb in range(4 * ci + 4, 4 * ci + 8):
                                    filler.append(lambda tb=tb: v_mm(tb))
                            if "d" in phases and ci > 0:
                                for tb in range(4 * ci - 4, 4 * ci):
                                    for nn_ in range(2):
                                        filler.append(
                                            lambda tb=tb, nn_=nn_:
                                            proj_block(tb, nn_))

                            # PV h1: zip fillers (QKV of ci+1 must drain here)
                            yt = yps.tile([128, 512], f32, tag="yt")
                            for p in range(npairs):
                                pv_pair(ci, 1, p, yt, es01[p][1])
                                fill(4)
                                if p == npairs // 2 and pend_norm is not None:
                                    norm(*pend_norm)
                                    pend_norm = None
                            if pend_norm is not None:
                                norm(*pend_norm)
                            pend_norm = (ci, 1, yt)
                            fill(len(filler) - 8 if ci + 1 < NI else 0)

                            # PV h2/h3: zip NEXT chunk's pair-(0,1) scores
                            es01_next = []
                            np_next = 2 * ci + 4
                            for h in (2, 3):
                                yt = yps.tile([128, 512], f32, tag="yt")
                                for p in range(npairs):
                                    if ci + 1 < NI and len(es01_next) < np_next:
                                        want = (np_next * (p + 1 +
                                                (h - 2) * npairs) +
                                                2 * npairs - 1) // (2 * npairs)
                                        while len(es01_next) < want:
                                            es01_next.append(
                                                s_halfpair(ci + 1, 0,
                                                           len(es01_next)))
                                    pv_pair(ci, h, p, yt, es23[p][h % 2])
                                    fill(2)
                                    if p == npairs // 2 and pend_norm is not None:
                                        norm(*pend_norm)
                                        pend_norm = None
                                if pend_norm is not None:
                                    norm(*pend_norm)
                                pend_norm = (ci, h, yt)
                            while ci + 1 < NI and len(es01_next) < np_next:
                                es01_next.append(
                                    s_halfpair(ci + 1, 0, len(es01_next)))
                            fill(len(filler))
                            es01 = es01_next
                        if pend_norm is not None:
                            norm(*pend_norm)
                        if "c" in phases and "d" in phases:
                            proj(NI - 1)

            if reps == 1:
                body()
            else:
                with tc.For_i(0, reps, 1, hint_engines=(
                        mybir.EngineType.PE, mybir.EngineType.Activation,
                        mybir.EngineType.DVE, mybir.EngineType.SP,
                        mybir.EngineType.Pool)):
                    body()

    if split_waits:
        split_excess_waits(nc)
    return nc


# ---------------------------------------------------------------------------
# Cached PJRT runner (fork of concourse.bass2jax.run_bass_via_pjrt that keeps
# the jitted executable so repeat kernel() calls don't recompile)
# ---------------------------------------------------------------------------
_RUNNERS = {}


def _make_pjrt(nc, donate=True, tag="main"):
    import jax
    from jax.sharding import Mesh, PartitionSpec
    from jax.experimental.shard_map import shard_map
    from concourse import bass2jax as b2j

    b2j.install_neuronx_cc_hook()

    partition_name = (
        nc.partition_id_tensor.name if nc.partition_id_tensor else None
    )
    in_names, out_names, out_avals, zero_outs = [], [], [], []
    for alloc in nc.m.functions[0].allocations:
        if not isinstance(alloc, mybir.MemoryLocationSet):
            continue
        name = alloc.memorylocations[0].name
        if alloc.kind == "ExternalInput":
            if name != partition_name:
                in_names.append(name)
        elif alloc.kind == "ExternalOutput":
            out_names.append(name)
            shape = tuple(alloc.tensor_shape)
            dtype = mybir.dt.np(alloc.dtype)
            out_avals.append(jax.core.ShapedArray(shape, dtype))
            zero_outs.append(np.zeros(shape, dtype))
    n_params = len(in_names)
    n_outs = len(out_avals)
    all_names = in_names + out_names
    if partition_name is not None:
        all_names = all_names + [partition_name]
    donate_idx = tuple(range(n_params, n_params + n_outs))

    def _body(*args):
        operands = list(args)
        if partition_name is not None:
            operands.append(b2j.partition_id_tensor())
        outs = b2j._bass_exec_p.bind(
            *operands,
            out_avals=tuple(out_avals),
            in_names=tuple(all_names),
            out_names=tuple(out_names),
            lowering_input_output_aliases=(),
            sim_require_finite=True,
            sim_require_nnan=True,
            nc=nc,
        )
        return tuple(outs)

    _body.__name__ = f"_body_{tag}"
    _body.__qualname__ = f"_body_{tag}"

    devices = jax.devices()[:N_CORES]
    mesh = Mesh(np.asarray(devices), ("core",))
    in_specs = (PartitionSpec("core"),) * (n_params + n_outs)
    out_specs = (PartitionSpec("core"),) * n_outs
    sharded = jax.jit(
        shard_map(_body, mesh=mesh, in_specs=in_specs, out_specs=out_specs,
                  check_rep=False),
        donate_argnums=donate_idx if donate else (), keep_unused=True)

    def concat_args(in_maps):
        per_core = [[np.asarray(m[name]) for name in in_names] for m in in_maps]
        concat_in = [
            np.concatenate([per_core[c][i] for c in range(N_CORES)], axis=0)
            for i in range(n_params)
        ]
        concat_zeros = [
            np.zeros((N_CORES * z.shape[0], *z.shape[1:]), z.dtype)
            for z in zero_outs
        ]
        return concat_in + concat_zeros

    def run(in_maps):
        out_arrs = sharded(*concat_args(in_maps))
        return [
            {name: np.asarray(out_arrs[i]).reshape(N_CORES, *out_avals[i].shape)[c]
             for i, name in enumerate(out_names)}
            for c in range(N_CORES)
        ]

    info = {
        "sharded": sharded, "concat_args": concat_args, "mesh": mesh,
        "PartitionSpec": PartitionSpec, "jax": jax,
    }
    return run, info


def _get_runner(key, nc):
    if key in _RUNNERS:
        return _RUNNERS[key]
    run, _ = _make_pjrt(nc, donate=True, tag=key)
    _RUNNERS[key] = run
    return run


def get_timed_runner(nc, tag="timed"):
    """No donation, device-resident args: returns (call, dev_args_fn)."""
    run, info = _make_pjrt(nc, donate=False, tag=tag)
    jax = info["jax"]
    sharding = jax.sharding.NamedSharding(
        info["mesh"], info["PartitionSpec"]("core"))

    def prepare(in_maps):
        return [jax.device_put(a, sharding) for a in info["concat_args"](in_maps)]

    def call(dev_args):
        outs = info["sharded"](*dev_args)
        jax.block_until_ready(outs)
        return outs

    return prepare, call


# ---------------------------------------------------------------------------
# Host-side sharding / gathering
# ---------------------------------------------------------------------------
def make_in_maps(x, W_attn, b_attn, W_proj):
    import ml_dtypes
    rj = np.arange(128)[:, None]
    ri = np.arange(128)[None, :]
    trimask = (rj <= ri).astype(ml_dtypes.bfloat16)
    in_maps = []
    for c in range(N_CORES):
        b = c // (N_CORES // B)
        g = c % (N_CORES // B)
        cs = slice(CPH * g, CPH * g + CPH)
        wq = W_attn[:, CPH * g:CPH * g + CPH]
        wk = W_attn[:, C + CPH * g:C + CPH * g + CPH]
        wv = W_attn[:, 2 * C + CPH * g:2 * C + CPH * g + CPH]
        wqkv = np.ascontiguousarray(
            np.concatenate([wq, wk, wv], axis=1).astype(ml_dtypes.bfloat16))
        bq = b_attn[CPH * g:CPH * g + CPH]
        bk = b_attn[C + CPH * g:C + CPH * g + CPH]
        bvv = b_attn[2 * C + CPH * g:2 * C + CPH * g + CPH]
        bqk = np.ascontiguousarray(
            np.stack([bq[:128], bq[128:], bk[:128], bk[128:]], axis=1))
        bv_arr = np.ascontiguousarray(
            np.broadcast_to(bvv[None, :], (128, CPH)))
        wp = np.ascontiguousarray(W_proj[cs, :].astype(ml_dtypes.bfloat16))
        in_maps.append({
            "xt": np.ascontiguousarray(x[b].T.astype(ml_dtypes.bfloat16)),
            "wqkv": wqkv, "bqk": bqk, "bv": bv_arr, "wp": wp,
            "trimask": trimask,
        })
    return in_maps


def kernel(x, W_attn, b_attn, W_proj, b_proj):
    x = np.asarray(x, dtype=np.float32)
    W_attn = np.asarray(W_attn, dtype=np.float32)
    b_attn = np.asarray(b_attn, dtype=np.float32)
    W_proj = np.asarray(W_proj, dtype=np.float32)
    b_proj = np.asarray(b_proj, dtype=np.float32)

    if "main" not in _RUNNERS:
        nc = build_program(reps=1)
        run = _get_runner("main", nc)
    else:
        run = _RUNNERS["main"]

    results = run(make_in_maps(x, W_attn, b_attn, W_proj))

    out = np.empty((B, T, C), dtype=np.float32)
    gpb = N_CORES // B
    for b in range(B):
        acc = results[gpb * b]["y"].astype(np.float32)
        for g in range(1, gpb):
            acc = acc + results[gpb * b + g]["y"].astype(np.float32)
        out[b] = acc + b_proj[None, :]
    return out


# revision 28
# speedup vs baseline: 1.0550x; 1.0272x over previous
"""Causal self-attention (B=2, T=2048, C=1024, H=16) on 8 TRN2 NeuronCores.

Sharding: data-parallel over batch x tensor-parallel over heads.
Core c handles batch c//4 and the 4 heads (c%4)*4 .. (c%4)*4+3:
  - QKV projection restricted to its heads' columns of W_attn
  - per-head causal attention (scores kept transposed: ST[j, i])
  - softmax denominator obtained by augmenting V with a ones column,
    so P@V and the row sums come from the same matmuls
  - row-parallel output projection with its heads' rows of W_proj
Host sums the 4 partial projections per batch and adds b_proj.

v2: x is pre-transposed on the host (no PE transposes / DVE copies).
The whole kernel is one modulo-software-pipelined stream over four
512-column chunks: during chunk ci's PV phase, h0 zips with ci's
pair-(2,3) scores, h1 zips the next chunk's QKV + previous chunk's
out-proj, and h2/h3 emit the NEXT chunk's pair-(0,1) scores, so the
Act engine (exp, the secondary bottleneck at ~0.85ns/col + 343ns/op)
never runs dry. Score tiles pack two j-blocks [128,1024] (diagonal
blocks compacted) to halve exp instruction count; the two heads of a
pair occupy PE row-quadrants 0:64/64:128 and their score matmuls are
interleaved for quadrant concurrency. Causal masking multiplies only
the 128x128 diagonal triangle in-place on Pool; softmax denominators
ride along as a ones-column of V; normalization is recip + ones-bcast
matmul; projection partials return as bf16 (halves output DMA).
"""
import os
import sys
sys.path.insert(0, '/opt/trn_rl_repo')
os.environ.setdefault("JAX_PLATFORMS", "axon,cpu")

from contextlib import ExitStack

import numpy as np

import concourse.bass as bass
import concourse.tile as tile
from concourse import library_config, mybir

B, T, C, H, HD = 2, 2048, 1024, 16, 64
N_CORES = 8
HPC = H // (N_CORES // B)     # heads per core = 4
CPH = HPC * HD                # channel slice per core = 256

f32 = mybir.dt.float32
f32r = mybir.dt.float32r
bf16 = mybir.dt.bfloat16
AF = mybir.ActivationFunctionType

# ---------------------------------------------------------------------------
# Workaround for this container's walrus codegen, which rejects instructions
# carrying more than one sync-wait command ("Too many sync wait commands").
# After Tile scheduling, hoist excess waits onto same-engine NoOps inserted
# immediately before the owning instruction (engine streams are sequential,
# so this preserves semantics exactly).
# ---------------------------------------------------------------------------
import concourse.tile as tile_mod
from bass_rust import ScopedClock, SyncInfo

MAX_WAITS = 1


def _drain_and_barrier(self, tick_clock, wait_clock):
    nc = self.nc
    drain_inst = nc.sync.drain()
    wait_clock.add_sem_waits(
        drain_inst.ins, ScopedClock({None: tick_clock.global_clock})
    )
    si = drain_inst.ins.sync_info
    if si is not None and len(si.on_wait) > MAX_WAITS:
        waits = list(si.on_wait)
        drain_inst.ins.sync_info = SyncInfo(
            on_wait=waits[:MAX_WAITS], on_update=list(si.on_update)
        )
        for k in range(MAX_WAITS, len(waits), MAX_WAITS):
            nop = nc.sync.nop(nofuse=True)
            nop.ins.sync_info = SyncInfo(on_wait=waits[k:k + MAX_WAITS], on_update=[])
    nc.all_engine_barrier()
    assert self.sems is not None
    popped = nc._tile_sem_poison_stack.pop()
    assert popped is self._sem_poison
    nc.clear_and_free_semaphores(list(self.sems.allocated().values()))
    nc.all_engine_barrier()


tile_mod.TileContext._drain_and_barrier = _drain_and_barrier

_split_counter = [0]


def split_excess_waits(nc, max_waits=MAX_WAITS):
    n_split = 0
    for f in nc.m.functions:
        for bb in f.blocks:
            il = bb.instructions
            out = []
            for ins in il:
                si = ins.sync_info
                if si is not None and len(si.on_wait) > max_waits:
                    waits = list(si.on_wait)
                    extra = waits[:-max_waits]
                    for k in range(0, len(extra), max_waits):
                        _split_counter[0] += 1
                        nop = mybir.InstNoOp(
                            name=f"wsplit-{_split_counter[0]}", ins=[], outs=[]
                        )
                        nop.engine = ins.engine
                        nop.sync_info = SyncInfo(
                            on_wait=extra[k:k + max_waits], on_update=[]
                        )
                        out.append(nop)
                    ins.sync_info = SyncInfo(
                        on_wait=waits[-max_waits:], on_update=list(si.on_update)
                    )
                    n_split += 1
                out.append(ins)
            if len(out) != len(il):
                il[:] = out
    return n_split


# ---------------------------------------------------------------------------
# Program builder
# ---------------------------------------------------------------------------
def build_program(reps=1, split_waits=True, phases="abcd"):
    nc = bass.Bass("TRN2", target_bir_lowering=False, debug=False)

    xt_d = nc.dram_tensor("xt", [C, T], bf16, kind="ExternalInput")
    wqkv_d = nc.dram_tensor("wqkv", [C, 3 * CPH], bf16, kind="ExternalInput")
    bqk_d = nc.dram_tensor("bqk", [128, 4], f32, kind="ExternalInput")
    bv_d = nc.dram_tensor("bv", [128, CPH], f32, kind="ExternalInput")
    wp_d = nc.dram_tensor("wp", [CPH, C], bf16, kind="ExternalInput")
    trimask_d = nc.dram_tensor("trimask", [128, 128], bf16, kind="ExternalInput")
    y_d = nc.dram_tensor("y", [T, C], bf16, kind="ExternalOutput")

    NT = T // 128    # 16 t-blocks
    NCB = C // 128   # 8 c-blocks
    NI = T // 512    # 4 i-chunks

    with tile.TileContext(nc) as tc:
        with ExitStack() as ctx:
            const = ctx.enter_context(tc.tile_pool(name="const", bufs=1))
            trimask_t = const.tile([128, 128], bf16, tag="trimask")
            nc.sync.dma_start(trimask_t[:], trimask_d.ap())
            bqk_t = const.tile([128, 4], f32, tag="bqk")
            nc.sync.dma_start(bqk_t[:], bqk_d.ap())
            bv_t = const.tile([128, CPH], f32, tag="bv")
            nc.sync.dma_start(bv_t[:], bv_d.ap())
            ones4_t = const.tile([128, 4], f32, tag="ones4")
            nc.gpsimd.memset(ones4_t[:], 1.0)
            tones_f = const.tile([1, 64], f32, tag="tones_f")
            nc.gpsimd.memset(tones_f[:], 1.0)
            tones_t = const.tile([1, 64], f32r, tag="tones")
            nc.vector.tensor_copy(tones_t[:], tones_f[:])

            def body():
                with ExitStack() as c2:
                    # ---- persistent SBUF -----------------------------------
                    xw_p = c2.enter_context(tc.tile_pool(name="xw", bufs=1))
                    qk_p = c2.enter_context(tc.tile_pool(name="qk", bufs=1))
                    va_p = c2.enter_context(tc.tile_pool(name="va", bufs=1))
                    yt_p = c2.enter_context(tc.tile_pool(name="yt", bufs=1))
                    xt = [xw_p.tile([128, T], bf16, tag=f"xt{cb}", name=f"xt{cb}")
                          for cb in range(NCB)]
                    wt = [xw_p.tile([128, 3 * CPH], bf16, tag=f"wt{cb}",
                                    name=f"wt{cb}") for cb in range(NCB)]
                    wpt = [xw_p.tile([128, C], bf16, tag=f"wp{kb}",
                                     name=f"wpt{kb}") for kb in range(2)]
                    # qkt[0..1]: Q^T two heads per tile; qkt[2..3]: K^T
                    qkt = [qk_p.tile([128, T], bf16, tag=f"qkt{m}", name=f"qkt{m}")
                           for m in range(4)]
                    # V augmented with a ones column per head: [128, 4*65]
                    vaug = [va_p.tile([128, HPC * 65], bf16, tag=f"va{tb}",
                                      name=f"va{tb}") for tb in range(NT)]
                    # normalized Y^T, two heads stacked per tile
                    yts = [yt_p.tile([128, T], bf16, tag=f"yts{k}", name=f"yts{k}")
                           for k in range(2)]

                    def dma_chunk(ci):
                        csl = slice(ci * 512, ci * 512 + 512)
                        for cb in range(NCB):
                            nc.sync.dma_start(
                                xt[cb][:, csl],
                                xt_d.ap()[cb * 128:(cb + 1) * 128, csl])

                    # interleave per-cb so the first QKV accumulation chain
                    # can start as soon as each (wt, xt) pair lands
                    for cb in range(NCB):
                        nc.sync.dma_start(
                            wt[cb][:, 0:384],
                            wqkv_d.ap()[cb * 128:(cb + 1) * 128, 0:384])
                        nc.sync.dma_start(
                            xt[cb][:, 0:512],
                            xt_d.ap()[cb * 128:(cb + 1) * 128, 0:512])
                    for cb in range(NCB):
                        nc.sync.dma_start(
                            wt[cb][:, 384:768],
                            wqkv_d.ap()[cb * 128:(cb + 1) * 128, 384:768])
                    for kb in range(2):
                        nc.sync.dma_start(wpt[kb][:],
                                          wp_d.ap()[kb * 128:(kb + 1) * 128, :])

                    with ExitStack() as c3:
                        genps = c3.enter_context(
                            tc.tile_pool(name="genps", bufs=2, space="PSUM"))
                        sps = c3.enter_context(
                            tc.tile_pool(name="sps", bufs=2, space="PSUM"))
                        yps = c3.enter_context(
                            tc.tile_pool(name="yps", bufs=2, space="PSUM"))
                        ep = c3.enter_context(tc.tile_pool(name="ep", bufs=20))
                        rp = c3.enter_context(tc.tile_pool(name="rp", bufs=8))
                        op = c3.enter_context(tc.tile_pool(name="op", bufs=8))

                        if "c" not in phases:
                            for k2 in range(2):
                                nc.vector.memset(yts[k2][:], 0.0)

                        # ---- emission helpers ------------------------------
                        def qk_mm(ci, m):
                            isl = slice(ci * 512, ci * 512 + 512)
                            ps = genps.tile([128, 512], f32, tag="gen")
                            for cb in range(NCB):
                                nc.tensor.matmul(
                                    ps[:],
                                    lhsT=wt[cb][:, m * 128:(m + 1) * 128],
                                    rhs=xt[cb][:, isl],
                                    start=(cb == 0), stop=(cb == NCB - 1))
                            nc.vector.tensor_scalar_add(
                                qkt[m][:, isl], ps[:], bqk_t[:, m:m + 1])

                        def v_mm(tb):
                            ps = genps.tile([128, 512], f32, tag="gen")
                            for cb in range(NCB):
                                nc.tensor.matmul(
                                    ps[:, 0:CPH],
                                    lhsT=xt[cb][:, tb * 128:(tb + 1) * 128],
                                    rhs=wt[cb][:, 2 * CPH:3 * CPH],
                                    start=(cb == 0), stop=(cb == NCB - 1))
                            vv = vaug[tb][:].rearrange("p (h e) -> p h e", e=65)
                            nc.vector.tensor_add(
                                vv[:, :, 0:64],
                                ps[:, 0:CPH].rearrange("p (h d) -> p h d", d=64),
                                bv_t[:].rearrange("p (h d) -> p h d", d=64))
                            nc.gpsimd.tensor_copy(
                                vv[:, :, 64:65],
                                ones4_t[:].rearrange("p (h e) -> p h e", e=1))

                        # j-block layout inside a score tile for logical
                        # pair index p of chunk ci: full pairs pack j-blocks
                        # (2p, 2p+1) at columns 0/512; the two diagonal
                        # "pairs" pack (4ci, 4ci+2) at columns 0/512 and
                        # (4ci+1, 4ci+3) at columns 0/384 (compacted so one
                        # exp covers the whole used range).
                        def pair_blocks(ci, p):
                            """[(bj, tile_col, lo), ...] for pair p; plus the
                            number of used columns."""
                            if p < 2 * ci:
                                return [(2 * p, 0, 0), (2 * p + 1, 512, 0)], 1024
                            if p == 2 * ci:      # diag A: k=0 and k=2
                                return [(4 * ci, 0, 0),
                                        (4 * ci + 2, 512, 256)], 768
                            # diag B: k=1 and k=3
                            return [(4 * ci + 1, 0, 128),
                                    (4 * ci + 3, 384, 384)], 512

                        def s_halfpair(ci, hp, p):
                            """Scores for head pair (2hp, 2hp+1), logical pair
                            p vs i-chunk ci. The two heads sit in row quadrants
                            0:64 / 64:128 of the qkt tiles, so alternating
                            their matmuls lets the PE run both quadrants
                            concurrently. Returns (es_lo, es_hi)."""
                            isl = slice(ci * 512, ci * 512 + 512)
                            qt_t = qkt[hp]
                            kt_t = qkt[2 + hp]
                            blocks, used = pair_blocks(ci, p)
                            sts, ess = [], []
                            for sub in range(2):
                                sts.append(sps.tile([128, 1024], f32, tag="st", name=f"st{sub}"))
                                ess.append(ep.tile([128, 1024], bf16, tag="es", name=f"es{sub}"))
                            for (bj, col, lo) in blocks:
                                jsl = slice(bj * 128, bj * 128 + 128)
                                osl = slice(col, col + 512 - lo)
                                for sub in range(2):   # alternate quadrants
                                    prow = slice(sub * 64, sub * 64 + 64)
                                    nc.tensor.matmul(
                                        sts[sub][:, osl],
                                        lhsT=kt_t[prow, jsl],
                                        rhs=qt_t[prow, isl][:, lo:],
                                        start=True, stop=True)
                            for sub in range(2):
                                st, es = sts[sub], ess[sub]
                                nc.scalar.activation(
                                    es[:, 0:used], st[:, 0:used], AF.Exp,
                                    scale=0.125)
                                if p >= 2 * ci:     # mask diag triangles
                                    for (bj, col, lo) in blocks:
                                        msl = slice(col, col + 128)
                                        nc.gpsimd.tensor_mul(
                                            es[:, msl], es[:, msl],
                                            trimask_t[:])
                            return ess

                        def pv_pair(ci, h, p, yt, es):
                            blocks, _ = pair_blocks(ci, p)
                            for (bj, col, lo) in blocks:
                                nc.tensor.matmul(
                                    yt[0:65, lo:],
                                    lhsT=vaug[bj][:, h * 65:(h + 1) * 65],
                                    rhs=es[:, col:col + 512 - lo],
                                    start=(bj == 0), stop=(bj == 4 * ci + 3))

                        def norm(ci, h, yt):
                            """recip + ones-bcast matmul + scale into yts."""
                            isl = slice(ci * 512, ci * 512 + 512)
                            prow = slice((h % 2) * 64, (h % 2) * 64 + 64)
                            rc = rp.tile([1, 512], f32r, tag="rc")
                            with nc.allow_low_precision(
                                    reason="f32r operand for bcast matmul"):
                                nc.vector.reciprocal(rc[:], yt[64:65, :])
                            bc = genps.tile([128, 512], f32, tag="gen")
                            nc.tensor.matmul(bc[0:64, :], lhsT=tones_t[:],
                                             rhs=rc[:], start=True, stop=True)
                            bs = rp.tile([64, 512], f32, tag="bs")
                            nc.vector.tensor_copy(bs[:], bc[0:64, :])
                            nc.vector.tensor_mul(
                                yts[h // 2][prow, isl], yt[0:64, :], bs[:])

                        def proj_block(tb, nn_):
                            ps = genps.tile([128, 512], f32, tag="gen")
                            for kb in range(2):
                                nc.tensor.matmul(
                                    ps[:],
                                    lhsT=yts[kb][:, tb * 128:(tb + 1) * 128],
                                    rhs=wpt[kb][:, nn_ * 512:(nn_ + 1) * 512],
                                    start=(kb == 0), stop=(kb == 1))
                            ob = op.tile([128, 512], bf16, tag="ob")
                            nc.vector.tensor_copy(ob[:], ps[:])
                            nc.sync.dma_start(
                                y_d.ap()[tb * 128:(tb + 1) * 128,
                                         nn_ * 512:(nn_ + 1) * 512],
                                ob[:])

                        def proj(ci):
                            for tb in range(4 * ci, 4 * ci + 4):
                                for nn_ in range(2):
                                    proj_block(tb, nn_)

                        # ---- interleaved emission --------------------------
                        # Modulo software pipeline across chunks. During chunk
                        # ci's PV phase: h0 zips with ci's pair-(2,3) scores,
                        # h1 zips next chunk's QKV + previous chunk's proj,
                        # h2/h3 zip the NEXT chunk's pair-(0,1) scores, so the
                        # Act engine (exp) never runs dry between chunks.
                        from collections import deque
                        filler = deque()

                        def fill(n):
                            for _ in range(min(n, len(filler))):
                                filler.popleft()()

                        pend_norm = None     # (ci, h, yt) awaiting emission
                        es01 = None
                        for ci in range(NI):
                            npairs = 2 * ci + 2
                            if ci + 1 < NI:
                                dma_chunk(ci + 1)
                            if "c" not in phases:
                                if ci == 0:
                                    for m in (0, 2, 1, 3):
                                        qk_mm(0, m)
                                    for tb in range(0, 4):
                                        v_mm(tb)
                                else:
                                    fill(len(filler))
                                if "d" in phases:
                                    proj(ci)
                                if ci + 1 < NI:
                                    for m in (0, 2, 1, 3):
                                        filler.append(
                                            lambda ci=ci, m=m: qk_mm(ci + 1, m))
                                    for tb in range(4 * ci + 4, 4 * ci + 8):
                                        filler.append(
                                            lambda tb=tb: v_mm(tb))
                                continue

                            if ci == 0:
                                # prologue: chunk 0 QKV + pair-(0,1) scores
                                qk_mm(0, 0)
                                qk_mm(0, 2)
                                es01 = []
                                for p in range(npairs):
                                    es01.append(s_halfpair(0, 0, p))
                                    if p == 0:
                                        qk_mm(0, 1)
                                        qk_mm(0, 3)
                                    v_mm(2 * p)
                                    v_mm(2 * p + 1)

                            # PV h0 zipped with this chunk's pair-(2,3) scores
                            es23 = []
                            yt0 = yps.tile([128, 512], f32, tag="yt")
                            for p in range(npairs):
                                es23.append(s_halfpair(ci, 1, p))
                                pv_pair(ci, 0, p, yt0, es01[p][0])
                                if p == 0 and pend_norm is not None:
                                    norm(*pend_norm)
                                    pend_norm = None
                            pend_norm = (ci, 0, yt0)

                            # queue fillers: next chunk's QKV, prev chunk proj
                            if ci + 1 < NI:
                                for m in (0, 2, 1, 3):
                                    filler.append(
                                        lambda ci=ci, m=m: qk_mm(ci + 1, m))
                                for tb in range(4 * ci + 4, 4 * ci + 8):
                                    filler.append(lambda tb=tb: v_mm(tb))
                            if "d" in phases and ci > 0:
                                for tb in range(4 * ci - 4, 4 * ci):
                                    for nn_ in range(2):
                                        filler.append(
                                            lambda tb=tb, nn_=nn_:
                                            proj_block(tb, nn_))

                            # PV h1: zip fillers (QKV of ci+1 must drain here)
                            yt = yps.tile([128, 512], f32, tag="yt")
                            for p in range(npairs):
                                pv_pair(ci, 1, p, yt, es01[p][1])
                                fill(4)
                                if p == npairs // 2 and pend_norm is not None:
                                    norm(*pend_norm)
                                    pend_norm = None
                            if pend_norm is not None:
                                norm(*pend_norm)
                            pend_norm = (ci, 1, yt)
                            fill(len(filler) - 8 if ci + 1 < NI else 0)

                            # PV h2/h3: zip NEXT chunk's pair-(0,1) scores
                            es01_next = []
                            np_next = 2 * ci + 4
                            for h in (2, 3):
                                yt = yps.tile([128, 512], f32, tag="yt")
                                for p in range(npairs):
                                    if ci + 1 < NI and len(es01_next) < np_next:
                                        want = (np_next * (p + 1 +
                                                (h - 2) * npairs) +
                                                2 * npairs - 1) // (2 * npairs)
                                        while len(es01_next) < want:
                                            es01_next.append(
                                                s_halfpair(ci + 1, 0,
                                                           len(es01_next)))
                                    pv_pair(ci, h, p, yt, es23[p][h % 2])
                                    fill(2)
                                    if p == npairs // 2 and pend_norm is not None:
                                        norm(*pend_norm)
                                        pend_norm = None
                                if pend_norm is not None:
                                    norm(*pend_norm)
                                pend_norm = (ci, h, yt)
                            while ci + 1 < NI and len(es01_next) < np_next:
                                es01_next.append(
                                    s_halfpair(ci + 1, 0, len(es01_next)))
                            fill(len(filler))
                            es01 = es01_next
                        if pend_norm is not None:
                            norm(*pend_norm)
                        if "c" in phases and "d" in phases:
                            proj(NI - 1)

            if reps == 1:
                body()
            else:
                with tc.For_i(0, reps, 1, hint_engines=(
                        mybir.EngineType.PE, mybir.EngineType.Activation,
                        mybir.EngineType.DVE, mybir.EngineType.SP,
                        mybir.EngineType.Pool)):
                    body()

    if split_waits:
        split_excess_waits(nc)
    return nc


# ---------------------------------------------------------------------------
# Cached PJRT runner (fork of concourse.bass2jax.run_bass_via_pjrt that keeps
# the jitted executable so repeat kernel() calls don't recompile)
# ---------------------------------------------------------------------------
_RUNNERS = {}


def _make_pjrt(nc, donate=True, tag="main"):
    import jax
    from jax.sharding import Mesh, PartitionSpec
    from jax.experimental.shard_map import shard_map
    from concourse import bass2jax as b2j

    b2j.install_neuronx_cc_hook()

    partition_name = (
        nc.partition_id_tensor.name if nc.partition_id_tensor else None
    )
    in_names, out_names, out_avals, zero_outs = [], [], [], []
    for alloc in nc.m.functions[0].allocations:
        if not isinstance(alloc, mybir.MemoryLocationSet):
            continue
        name = alloc.memorylocations[0].name
        if alloc.kind == "ExternalInput":
            if name != partition_name:
                in_names.append(name)
        elif alloc.kind == "ExternalOutput":
            out_names.append(name)
            shape = tuple(alloc.tensor_shape)
            dtype = mybir.dt.np(alloc.dtype)
            out_avals.append(jax.core.ShapedArray(shape, dtype))
            zero_outs.append(np.zeros(shape, dtype))
    n_params = len(in_names)
    n_outs = len(out_avals)
    all_names = in_names + out_names
    if partition_name is not None:
        all_names = all_names + [partition_name]
    donate_idx = tuple(range(n_params, n_params + n_outs))

    def _body(*args):
        operands = list(args)
        if partition_name is not None:
            operands.append(b2j.partition_id_tensor())
        outs = b2j._bass_exec_p.bind(
            *operands,
            out_avals=tuple(out_avals),
            in_names=tuple(all_names),
            out_names=tuple(out_names),
            lowering_input_output_aliases=(),
            sim_require_finite=True,
            sim_require_nnan=True,
            nc=nc,
        )
        return tuple(outs)

    _body.__name__ = f"_body_{tag}"
    _body.__qualname__ = f"_body_{tag}"

    devices = jax.devices()[:N_CORES]
    mesh = Mesh(np.asarray(devices), ("core",))
    in_specs = (PartitionSpec("core"),) * (n_params + n_outs)
    out_specs = (PartitionSpec("core"),) * n_outs
    sharded = jax.jit(
        shard_map(_body, mesh=mesh, in_specs=in_specs, out_specs=out_specs,
                  check_rep=False),
        donate_argnums=donate_idx if donate else (), keep_unused=True)

    def concat_args(in_maps):
        per_core = [[np.asarray(m[name]) for name in in_names] for m in in_maps]
        concat_in = [
            np.concatenate([per_core[c][i] for c in range(N_CORES)], axis=0)
            for i in range(n_params)
        ]
        concat_zeros = [
            np.zeros((N_CORES * z.shape[0], *z.shape[1:]), z.dtype)
            for z in zero_outs
        ]
        return concat_in + concat_zeros

    def run(in_maps):
        out_arrs = sharded(*concat_args(in_maps))
        return [
            {name: np.asarray(out_arrs[i]).reshape(N_CORES, *out_avals[i].shape)[c]
             for i, name in enumerate(out_names)}
            for c in range(N_CORES)
        ]

    info = {
        "sharded": sharded, "concat_args": concat_args, "mesh": mesh,
        "PartitionSpec": PartitionSpec, "jax": jax,
    }
    return run, info


def _get_runner(key, nc):
    if key in _RUNNERS:
        return _RUNNERS[key]
    run, _ = _make_pjrt(nc, donate=True, tag=key)
    _RUNNERS[key] = run
    return run


def get_timed_runner(nc, tag="timed"):
    """No donation, device-resident args: returns (call, dev_args_fn)."""
    run, info = _make_pjrt(nc, donate=False, tag=tag)
    jax = info["jax"]
    sharding = jax.sharding.NamedSharding(
        info["mesh"], info["PartitionSpec"]("core"))

    def prepare(in_maps):
        return [jax.device_put(a, sharding) for a in info["concat_args"](in_maps)]

    def call(dev_args):
        outs = info["sharded"](*dev_args)
        jax.block_until_ready(outs)
        return outs

    return prepare, call


# ---------------------------------------------------------------------------
# Host-side sharding / gathering
# ---------------------------------------------------------------------------
def make_in_maps(x, W_attn, b_attn, W_proj):
    import ml_dtypes
    rj = np.arange(128)[:, None]
    ri = np.arange(128)[None, :]
    trimask = (rj <= ri).astype(ml_dtypes.bfloat16)
    in_maps = []
    for c in range(N_CORES):
        b = c // (N_CORES // B)
        g = c % (N_CORES // B)
        cs = slice(CPH * g, CPH * g + CPH)
        wq = W_attn[:, CPH * g:CPH * g + CPH]
        wk = W_attn[:, C + CPH * g:C + CPH * g + CPH]
        wv = W_attn[:, 2 * C + CPH * g:2 * C + CPH * g + CPH]
        wqkv = np.ascontiguousarray(
            np.concatenate([wq, wk, wv], axis=1).astype(ml_dtypes.bfloat16))
        bq = b_attn[CPH * g:CPH * g + CPH]
        bk = b_attn[C + CPH * g:C + CPH * g + CPH]
        bvv = b_attn[2 * C + CPH * g:2 * C + CPH * g + CPH]
        bqk = np.ascontiguousarray(
            np.stack([bq[:128], bq[128:], bk[:128], bk[128:]], axis=1))
        bv_arr = np.ascontiguousarray(
            np.broadcast_to(bvv[None, :], (128, CPH)))
        wp = np.ascontiguousarray(W_proj[cs, :].astype(ml_dtypes.bfloat16))
        in_maps.append({
            "xt": np.ascontiguousarray(x[b].T.astype(ml_dtypes.bfloat16)),
            "wqkv": wqkv, "bqk": bqk, "bv": bv_arr, "wp": wp,
            "trimask": trimask,
        })
    return in_maps


def kernel(x, W_attn, b_attn, W_proj, b_proj):
    x = np.asarray(x, dtype=np.float32)
    W_attn = np.asarray(W_attn, dtype=np.float32)
    b_attn = np.asarray(b_attn, dtype=np.float32)
    W_proj = np.asarray(W_proj, dtype=np.float32)
    b_proj = np.asarray(b_proj, dtype=np.float32)

    if "main" not in _RUNNERS:
        nc = build_program(reps=1)
        run = _get_runner("main", nc)
    else:
        run = _RUNNERS["main"]

    results = run(make_in_maps(x, W_attn, b_attn, W_proj))

    out = np.empty((B, T, C), dtype=np.float32)
    gpb = N_CORES // B
    for b in range(B):
        acc = results[gpb * b]["y"].astype(np.float32)
        for g in range(1, gpb):
            acc = acc + results[gpb * b + g]["y"].astype(np.float32)
        out[b] = acc + b_proj[None, :]
    return out
